# revision 12
# baseline (speedup 1.0000x reference)
"""nn_GateMulti — MoE routing (8 experts, one-hot gate) on 8 TRN2 NeuronCores.

Strategy: expert-parallel. The gate is exactly one-hot on groups[:, 0], so
each token needs exactly one expert's MLP. Host-side "all-to-all": sort the
4096 tokens by expert id, pad each expert's token set to a common capacity,
and hand core e exactly expert e's tokens (transposed) plus expert e's
weights. Each core then runs a dense 2-layer MLP:

    yT = W2.T @ relu(W1.T @ xT + b1) + b2        (feature-major layout)

Moving operands (xT, hT) are float32r (FP22-truncated fp32) which streams at
full PE rate for moving dims >= 256; stationary weights are bf16 (halves
weight DMA, enables fast-weight-load). The host scatters per-core outputs
back to the original token order. Compute per core is ~cap/512 of the ideal
balanced load (cap = max tokens routed to one expert), ~8x less than the
dense all-experts reference.

Weights are host-packed into the exact SBUF image layout so every weight DMA
moves multi-KB contiguous lines; w1 streams on the sync HWDGE ring while w2
streams on the scalar HWDGE ring.

Self-contained: shapes hardcoded from the problem spec.
"""

import math
from functools import lru_cache

import ml_dtypes
import numpy as np

import concourse.bacc as bacc
import concourse.mybir as mybir
import concourse.tile as tile
from concourse.bass_utils import run_bass_kernel_spmd

E = 8
B = 4096
D_IN = 512
D_FF = 2048
D_OUT = 512
GROUP_COL = 0

P = 128
D_T = D_IN // P   # 4  k-tiles for layer 1
F_T = D_FF // P   # 16 f-tiles (layer-1 out / layer-2 contraction)
O_T = D_OUT // P  # 4  o-tiles for layer 2
W1_G = 4          # w1 DMA granularity: f-tiles per DMA (slice 0 leads the ring)

F32 = mybir.dt.float32
F32R = mybir.dt.float32r
BF16 = mybir.dt.bfloat16

# walrus requires matmul operand dtypes to match when fp32/fp32r is involved,
# so weights and moving activations share one dtype: bf16 (fast weight load,
# half the DMA bytes) or float32r (better precision, full PE rate at N>=256).
ALL_BF16 = True
W_DT = A_DT = BF16 if ALL_BF16 else F32R
W_NP = ml_dtypes.bfloat16 if ALL_BF16 else np.float32


N_WARM = 18  # dependency-free scratch matmuls to lift the PE HAM clock
             # gate to 8/8 (2.4 GHz) while the pilot DMAs are in flight


def _emit(tc, nc, xT, w1, w2, b1t, b2t, yT, cap, n_chunks, chunk):
    relu = mybir.ActivationFunctionType.Relu
    ident = mybir.ActivationFunctionType.Identity
    from concourse.bass import _add_dep_helper

    with (
        tc.tile_pool(name="consts", bufs=1) as cpool,
        tc.tile_pool(name="acts", bufs=1) as apool,
        tc.tile_pool(name="yout", bufs=4) as ypool,
        tc.tile_pool(name="psum_h", bufs=4, space="PSUM") as ph,
        tc.tile_pool(name="psum_y", bufs=4, space="PSUM") as py,
    ):
        # ---- PE warm-up: scratch matmuls with no input dependencies. They
        # run during the input-DMA wait and hold the HAM activity window
        # busy, so the real stream starts at 2.4 GHz instead of 1.2 GHz.
        warm_w = cpool.tile([P, P], W_DT)
        warm_x = cpool.tile([P, chunk], A_DT)
        nc.gpsimd.memset(warm_w[:], 0.0)
        nc.gpsimd.memset(warm_x[:], 0.0)
        warm_p = py.tile([P, chunk], F32, name="warm_p", tag="yp")
        for _ in range(N_WARM):
            nc.tensor.matmul(warm_p[:], warm_w[:], warm_x[:])

        # ---- input DMAs. Within one HWDGE ring, DMAs complete in issue
        # order; the two rings fair-share the SDMA engines per packet (by
        # packet size, so all big transfers use multi-KB lines). Sync queue:
        # w1 first i-slice (smallest piece the first matmul needs), rest of
        # w1, then w2 gated behind the first real matmul (sync is otherwise
        # idle, so the gate blocks nothing). Scalar queue: xT + b1 then the
        # relu/bias ACTs -- no gated work may sit ahead of the ACTs, else
        # PSUM drain stalls the PE.
        w1_sb = cpool.tile([P, F_T, D_T, P], W_DT)   # [p, i, j, c]
        w2_sb = cpool.tile([P, O_T, F_T, P], W_DT)   # [p, k, i, c]
        xT_sb = apool.tile([P, D_T, cap], A_DT)

        for lo, hi in [(0, 1), (1, 5), (5, 9), (9, 13), (13, 16)]:
            nc.sync.dma_start(w1_sb[:, lo:hi], w1.ap()[:, lo:hi])
        b1_sb = cpool.tile([P, F_T], F32)
        nc.scalar.dma_start(b1_sb[:], b1t.ap())
        for c in range(n_chunks):
            cs = slice(c * chunk, (c + 1) * chunk)
            nc.scalar.dma_start(xT_sb[:, :, cs], xT.ap()[:, :, cs])
        bulk = []
        for k in range(O_T):
            bulk.append(nc.sync.dma_start(w2_sb[:, k], w2.ap()[:, k]))
        b2_sb = cpool.tile([P, O_T], F32)
        bulk.append(nc.sync.dma_start(b2_sb[:], b2t.ap()))

        hT_sb = apool.tile([P, F_T, cap], A_DT)

        first_mm = None
        # ---- layer 1: hT[f, c] = relu(sum_d W1[d, f] xT[d, c] + b1[f])
        # chunk-interleaved so each w1 i-slice is consumed over 2x the time
        for i in range(F_T):
            for c in range(n_chunks):
                cs = slice(c * chunk, (c + 1) * chunk)
                hp = ph.tile([P, chunk], F32, name=f"hp_{i}_{c}", tag="hp")
                for j in range(D_T):
                    mm = nc.tensor.matmul(
                        hp[:],
                        w1_sb[:, i, j, :],
                        xT_sb[:, j, cs],
                        start=(j == 0),
                        stop=(j == D_T - 1),
                    )
                    if first_mm is None:
                        first_mm = mm
                nc.scalar.activation(
                    hT_sb[:, i, cs], hp[:], relu, bias=b1_sb[:, i : i + 1]
                )
        for d in bulk:
            _add_dep_helper(d.ins, first_mm.ins, sync=True, reason="hold w2 until pilot set landed")
        # ---- layer 2: yT[o, c] = sum_f W2[f, o] hT[f, c] + b2[o]
        # output DMAs split across both HWDGE rings to shorten the tail
        for k in range(O_T):
            for c in range(n_chunks):
                cs = slice(c * chunk, (c + 1) * chunk)
                yp = py.tile([P, chunk], F32, name=f"yp_{k}_{c}", tag="yp")
                for i in range(F_T):
                    nc.tensor.matmul(
                        yp[:],
                        w2_sb[:, k, i, :],
                        hT_sb[:, i, cs],
                        start=(i == 0),
                        stop=(i == F_T - 1),
                    )
                yo = ypool.tile([P, chunk], F32, name=f"yo_{k}_{c}", tag="yo")
                half = chunk // 2
                c0 = c * chunk
                last = k == O_T - 1 and c == n_chunks - 1
                if not last:
                    nc.scalar.activation(yo[:], yp[:], ident, bias=b2_sb[:, k : k + 1])
                    nc.sync.dma_start(
                        yT[k * P : (k + 1) * P, c0 : c0 + half], yo[:, 0:half]
                    )
                    nc.scalar.dma_start(
                        yT[k * P : (k + 1) * P, c0 + half : c0 + chunk],
                        yo[:, half:chunk],
                    )
                else:
                    # final group: two half-width ACTs so the last bytes can
                    # leave (on both rings) as early as possible
                    nc.scalar.activation(
                        yo[:, 0:half], yp[:, 0:half], ident, bias=b2_sb[:, k : k + 1]
                    )
                    nc.sync.dma_start(
                        yT[k * P : (k + 1) * P, c0 : c0 + half], yo[:, 0:half]
                    )
                    nc.scalar.activation(
                        yo[:, half:chunk], yp[:, half:chunk], ident,
                        bias=b2_sb[:, k : k + 1],
                    )
                    nc.scalar.dma_start(
                        yT[k * P : (k + 1) * P, c0 + half : c0 + chunk],
                        yo[:, half:chunk],
                    )


@lru_cache(maxsize=4)
def _build_nc(cap, n_chunks, chunk):
    nc = bacc.Bacc("TRN2", target_bir_lowering=False, debug=False, num_devices=E)
    xT = nc.dram_tensor("xT", [P, D_T, cap], A_DT, kind="ExternalInput")
    w1 = nc.dram_tensor("w1", [P, F_T, D_T, P], W_DT, kind="ExternalInput")
    w2 = nc.dram_tensor("w2", [P, O_T, F_T, P], W_DT, kind="ExternalInput")
    b1t = nc.dram_tensor("b1t", [P, F_T], F32, kind="ExternalInput")
    b2t = nc.dram_tensor("b2t", [P, O_T], F32, kind="ExternalInput")
    yT = nc.dram_tensor("yT", [D_OUT, cap], F32, kind="ExternalOutput")
    with tile.TileContext(nc) as tc:
        _emit(tc, nc, xT, w1, w2, b1t, b2t, yT, cap, n_chunks, chunk)
    nc.compile()
    return nc


def _plan_capacity(max_count):
    cap0 = max(int(max_count), 16)
    n_chunks = max(1, math.ceil(cap0 / 512))
    chunk = math.ceil(cap0 / (n_chunks * 2)) * 2
    return n_chunks * chunk, n_chunks, chunk


def _pack_w1(W1e):
    # w1img[p, i, j, c] = W1e[j*128 + p, i*128 + c]
    return np.ascontiguousarray(
        W1e.reshape(D_T, P, F_T, P).transpose(1, 2, 0, 3).astype(W_NP)
    )


def _pack_w2(W2e):
    # w2img[p, k, i, c] = W2e[i*128 + p, k*128 + c]
    return np.ascontiguousarray(
        W2e.reshape(F_T, P, O_T, P).transpose(1, 2, 0, 3).astype(W_NP)
    )


def _shard(x, groups, W1, b1, W2, b2):
    idx = np.asarray(groups)[:, GROUP_COL].astype(np.int64)
    order = np.argsort(idx, kind="stable")
    counts = np.bincount(idx, minlength=E)
    cap, n_chunks, chunk = _plan_capacity(counts.max())
    offs = np.concatenate([[0], np.cumsum(counts)])

    x = np.asarray(x, dtype=np.float32)
    W1 = np.asarray(W1, dtype=np.float32)
    b1 = np.asarray(b1, dtype=np.float32)
    W2 = np.asarray(W2, dtype=np.float32)
    b2 = np.asarray(b2, dtype=np.float32)

    in_maps, tok_ids = [], []
    for e in range(E):
        ids = order[offs[e] : offs[e + 1]]
        tok_ids.append(ids)
        xT = np.zeros((D_IN, cap), np.float32)
        xT[:, : len(ids)] = x[ids].T
        # pack to the SBUF image [p, j, c] so DMA lines are 4KB contiguous
        xT = np.ascontiguousarray(
            xT.reshape(D_T, P, cap).transpose(1, 0, 2).astype(W_NP)
        )
        in_maps.append(
            {
                "xT": xT,
                "w1": _pack_w1(W1[e]),
                "w2": _pack_w2(W2[e]),
                "b1t": np.ascontiguousarray(b1[e].reshape(F_T, P).T),
                "b2t": np.ascontiguousarray(b2[e].reshape(O_T, P).T),
            }
        )
    return in_maps, tok_ids, counts, cap, n_chunks, chunk


def _run(x, groups, W1, b1, W2, b2, trace=False, **spmd_kwargs):
    in_maps, tok_ids, counts, cap, n_chunks, chunk = _shard(x, groups, W1, b1, W2, b2)
    nc = _build_nc(cap, n_chunks, chunk)
    res = run_bass_kernel_spmd(
        nc, in_maps, core_ids=list(range(E)), trace=trace, **spmd_kwargs
    )
    out = np.zeros((B, D_OUT), np.float32)
    for e in range(E):
        yTe = res.results[e]["yT"]
        out[tok_ids[e]] = yTe[:, : counts[e]].T
    return out, res


def kernel(x, groups, W1, b1, W2, b2):
    out, _ = _run(x, groups, W1, b1, W2, b2)
    return out


# revision 13
# speedup vs baseline: 1.0408x; 1.0408x over previous
"""nn_GateMulti — MoE routing (8 experts, one-hot gate) on 8 TRN2 NeuronCores.

Strategy: expert-parallel. The gate is exactly one-hot on groups[:, 0], so
each token needs exactly one expert's MLP. Host-side "all-to-all": sort the
4096 tokens by expert id, pad each expert's token set to a common capacity,
and hand core e exactly expert e's tokens (transposed) plus expert e's
weights. Each core then runs a dense 2-layer MLP:

    yT = W2.T @ relu(W1.T @ xT + b1) + b2        (feature-major layout)

Moving operands (xT, hT) are float32r (FP22-truncated fp32) which streams at
full PE rate for moving dims >= 256; stationary weights are bf16 (halves
weight DMA, enables fast-weight-load). The host scatters per-core outputs
back to the original token order. Compute per core is ~cap/512 of the ideal
balanced load (cap = max tokens routed to one expert), ~8x less than the
dense all-experts reference.

Weights are host-packed into the exact SBUF image layout so every weight DMA
moves multi-KB contiguous lines; w1 streams on the sync HWDGE ring while w2
streams on the scalar HWDGE ring.

Self-contained: shapes hardcoded from the problem spec.
"""

import math
from functools import lru_cache

import ml_dtypes
import numpy as np

import concourse.bacc as bacc
import concourse.mybir as mybir
import concourse.tile as tile
from concourse.bass_utils import run_bass_kernel_spmd

E = 8
B = 4096
D_IN = 512
D_FF = 2048
D_OUT = 512
GROUP_COL = 0

P = 128
D_T = D_IN // P   # 4  k-tiles for layer 1
F_T = D_FF // P   # 16 f-tiles (layer-1 out / layer-2 contraction)
O_T = D_OUT // P  # 4  o-tiles for layer 2
W1_G = 4          # w1 DMA granularity: f-tiles per DMA (slice 0 leads the ring)

F32 = mybir.dt.float32
F32R = mybir.dt.float32r
BF16 = mybir.dt.bfloat16

# walrus requires matmul operand dtypes to match when fp32/fp32r is involved,
# so weights and moving activations share one dtype: bf16 (fast weight load,
# half the DMA bytes) or float32r (better precision, full PE rate at N>=256).
ALL_BF16 = True
W_DT = A_DT = BF16 if ALL_BF16 else F32R
W_NP = ml_dtypes.bfloat16 if ALL_BF16 else np.float32


N_WARM = 18  # dependency-free scratch matmuls to lift the PE HAM clock
             # gate to 8/8 (2.4 GHz) while the pilot DMAs are in flight


def _emit(tc, nc, xT, w1, w2, b1t, b2t, yT, cap, n_chunks, chunk):
    relu = mybir.ActivationFunctionType.Relu
    ident = mybir.ActivationFunctionType.Identity
    from concourse.bass import _add_dep_helper

    with (
        tc.tile_pool(name="consts", bufs=1) as cpool,
        tc.tile_pool(name="acts", bufs=1) as apool,
        tc.tile_pool(name="yout", bufs=4) as ypool,
        tc.tile_pool(name="psum_h", bufs=4, space="PSUM") as ph,
        tc.tile_pool(name="psum_y", bufs=4, space="PSUM") as py,
    ):
        # ---- PE warm-up: scratch matmuls with no input dependencies. They
        # run during the input-DMA wait and hold the HAM activity window
        # busy, so the real stream starts at 2.4 GHz instead of 1.2 GHz.
        warm_w = cpool.tile([P, P], W_DT)
        warm_x = cpool.tile([P, chunk], A_DT)
        nc.gpsimd.memset(warm_w[:], 0.0)
        nc.gpsimd.memset(warm_x[:], 0.0)
        warm_p = py.tile([P, chunk], F32, name="warm_p", tag="yp")
        for _ in range(N_WARM):
            nc.tensor.matmul(warm_p[:], warm_w[:], warm_x[:])

        # ---- input DMAs. Within one HWDGE ring, DMAs complete in issue
        # order; the two rings fair-share the SDMA engines per packet (by
        # packet size, so all big transfers use multi-KB lines). Sync queue:
        # w1 first i-slice (smallest piece the first matmul needs), rest of
        # w1, then w2 gated behind the first real matmul (sync is otherwise
        # idle, so the gate blocks nothing). Scalar queue: xT + b1 then the
        # relu/bias ACTs -- no gated work may sit ahead of the ACTs, else
        # PSUM drain stalls the PE.
        w1_sb = cpool.tile([P, F_T, D_T, P], W_DT)   # [p, i, j, c]
        w2_sb = cpool.tile([P, O_T, F_T, P], W_DT)   # [p, k, i, c]
        xT_sb = apool.tile([P, n_chunks, D_T, chunk], A_DT)

        for lo, hi in [(0, 1), (1, 5), (5, 9), (9, 13), (13, 16)]:
            nc.sync.dma_start(w1_sb[:, lo:hi], w1.ap()[:, lo:hi])
        b1_sb = cpool.tile([P, F_T], F32)
        nc.scalar.dma_start(b1_sb[:], b1t.ap())
        for c in range(n_chunks):
            nc.scalar.dma_start(xT_sb[:, c], xT.ap()[c])
        bulk = []
        for k in range(O_T):
            bulk.append(nc.sync.dma_start(w2_sb[:, k], w2.ap()[:, k]))
        b2_sb = cpool.tile([P, O_T], F32)
        bulk.append(nc.sync.dma_start(b2_sb[:], b2t.ap()))

        hT_sb = apool.tile([P, F_T, cap], A_DT)

        first_mm = None
        # ---- layer 1: hT[f, c] = relu(sum_d W1[d, f] xT[d, c] + b1[f])
        # chunk-interleaved so each w1 i-slice is consumed over 2x the time
        for i in range(F_T):
            for c in range(n_chunks):
                cs = slice(c * chunk, (c + 1) * chunk)
                hp = ph.tile([P, chunk], F32, name=f"hp_{i}_{c}", tag="hp")
                for j in range(D_T):
                    mm = nc.tensor.matmul(
                        hp[:],
                        w1_sb[:, i, j, :],
                        xT_sb[:, c, j, :],
                        start=(j == 0),
                        stop=(j == D_T - 1),
                    )
                    if first_mm is None:
                        first_mm = mm
                nc.scalar.activation(
                    hT_sb[:, i, cs], hp[:], relu, bias=b1_sb[:, i : i + 1]
                )
        for d in bulk:
            _add_dep_helper(d.ins, first_mm.ins, sync=True, reason="hold w2 until pilot set landed")
        # ---- layer 2: yT[o, c] = sum_f W2[f, o] hT[f, c] + b2[o]
        # output DMAs split across both HWDGE rings to shorten the tail
        for k in range(O_T):
            for c in range(n_chunks):
                cs = slice(c * chunk, (c + 1) * chunk)
                yp = py.tile([P, chunk], F32, name=f"yp_{k}_{c}", tag="yp")
                for i in range(F_T):
                    nc.tensor.matmul(
                        yp[:],
                        w2_sb[:, k, i, :],
                        hT_sb[:, i, cs],
                        start=(i == 0),
                        stop=(i == F_T - 1),
                    )
                yo = ypool.tile([P, chunk], F32, name=f"yo_{k}_{c}", tag="yo")
                half = chunk // 2
                c0 = c * chunk
                last = k == O_T - 1 and c == n_chunks - 1
                if not last:
                    nc.scalar.activation(yo[:], yp[:], ident, bias=b2_sb[:, k : k + 1])
                    nc.sync.dma_start(
                        yT[k * P : (k + 1) * P, c0 : c0 + half], yo[:, 0:half]
                    )
                    nc.scalar.dma_start(
                        yT[k * P : (k + 1) * P, c0 + half : c0 + chunk],
                        yo[:, half:chunk],
                    )
                else:
                    # final group: two half-width ACTs so the last bytes can
                    # leave (on both rings) as early as possible
                    nc.scalar.activation(
                        yo[:, 0:half], yp[:, 0:half], ident, bias=b2_sb[:, k : k + 1]
                    )
                    nc.sync.dma_start(
                        yT[k * P : (k + 1) * P, c0 : c0 + half], yo[:, 0:half]
                    )
                    nc.scalar.activation(
                        yo[:, half:chunk], yp[:, half:chunk], ident,
                        bias=b2_sb[:, k : k + 1],
                    )
                    nc.scalar.dma_start(
                        yT[k * P : (k + 1) * P, c0 + half : c0 + chunk],
                        yo[:, half:chunk],
                    )


@lru_cache(maxsize=4)
def _build_nc(cap, n_chunks, chunk):
    nc = bacc.Bacc("TRN2", target_bir_lowering=False, debug=False, num_devices=E)
    xT = nc.dram_tensor("xT", [n_chunks, P, D_T, chunk], A_DT, kind="ExternalInput")
    w1 = nc.dram_tensor("w1", [P, F_T, D_T, P], W_DT, kind="ExternalInput")
    w2 = nc.dram_tensor("w2", [P, O_T, F_T, P], W_DT, kind="ExternalInput")
    b1t = nc.dram_tensor("b1t", [P, F_T], F32, kind="ExternalInput")
    b2t = nc.dram_tensor("b2t", [P, O_T], F32, kind="ExternalInput")
    yT = nc.dram_tensor("yT", [D_OUT, cap], F32, kind="ExternalOutput")
    with tile.TileContext(nc) as tc:
        _emit(tc, nc, xT, w1, w2, b1t, b2t, yT, cap, n_chunks, chunk)
    nc.compile()
    return nc


def _plan_capacity(max_count):
    cap0 = max(int(max_count), 16)
    n_chunks = max(1, math.ceil(cap0 / 512))
    chunk = math.ceil(cap0 / (n_chunks * 2)) * 2
    return n_chunks * chunk, n_chunks, chunk


def _pack_w1(W1e):
    # w1img[p, i, j, c] = W1e[j*128 + p, i*128 + c]
    return np.ascontiguousarray(
        W1e.reshape(D_T, P, F_T, P).transpose(1, 2, 0, 3).astype(W_NP)
    )


def _pack_w2(W2e):
    # w2img[p, k, i, c] = W2e[i*128 + p, k*128 + c]
    return np.ascontiguousarray(
        W2e.reshape(F_T, P, O_T, P).transpose(1, 2, 0, 3).astype(W_NP)
    )


def _shard(x, groups, W1, b1, W2, b2):
    idx = np.asarray(groups)[:, GROUP_COL].astype(np.int64)
    order = np.argsort(idx, kind="stable")
    counts = np.bincount(idx, minlength=E)
    cap, n_chunks, chunk = _plan_capacity(counts.max())
    offs = np.concatenate([[0], np.cumsum(counts)])

    x = np.asarray(x, dtype=np.float32)
    W1 = np.asarray(W1, dtype=np.float32)
    b1 = np.asarray(b1, dtype=np.float32)
    W2 = np.asarray(W2, dtype=np.float32)
    b2 = np.asarray(b2, dtype=np.float32)

    in_maps, tok_ids = [], []
    for e in range(E):
        ids = order[offs[e] : offs[e + 1]]
        tok_ids.append(ids)
        xT = np.zeros((D_IN, cap), np.float32)
        xT[:, : len(ids)] = x[ids].T
        # pack each chunk to its SBUF image [p, j, c] so every chunk DMA
        # moves multi-KB contiguous lines on both sides
        xT = np.ascontiguousarray(
            xT.reshape(D_T, P, n_chunks, chunk)
            .transpose(2, 1, 0, 3)
            .astype(W_NP)
        )
        in_maps.append(
            {
                "xT": xT,
                "w1": _pack_w1(W1[e]),
                "w2": _pack_w2(W2[e]),
                "b1t": np.ascontiguousarray(b1[e].reshape(F_T, P).T),
                "b2t": np.ascontiguousarray(b2[e].reshape(O_T, P).T),
            }
        )
    return in_maps, tok_ids, counts, cap, n_chunks, chunk


def _run(x, groups, W1, b1, W2, b2, trace=False, **spmd_kwargs):
    in_maps, tok_ids, counts, cap, n_chunks, chunk = _shard(x, groups, W1, b1, W2, b2)
    nc = _build_nc(cap, n_chunks, chunk)
    res = run_bass_kernel_spmd(
        nc, in_maps, core_ids=list(range(E)), trace=trace, **spmd_kwargs
    )
    out = np.zeros((B, D_OUT), np.float32)
    for e in range(E):
        yTe = res.results[e]["yT"]
        out[tok_ids[e]] = yTe[:, : counts[e]].T
    return out, res


def kernel(x, groups, W1, b1, W2, b2):
    out, _ = _run(x, groups, W1, b1, W2, b2)
    return out


# revision 14
# speedup vs baseline: 1.0731x; 1.0310x over previous
"""nn_GateMulti — MoE routing (8 experts, one-hot gate) on 8 TRN2 NeuronCores.

Strategy: expert-parallel. The gate is exactly one-hot on groups[:, 0], so
each token needs exactly one expert's MLP. Host-side "all-to-all": sort the
4096 tokens by expert id, pad each expert's token set to a common capacity,
and hand core e exactly expert e's tokens (transposed) plus expert e's
weights. Each core then runs a dense 2-layer MLP:

    yT = W2.T @ relu(W1.T @ xT + b1) + b2        (feature-major layout)

Moving operands (xT, hT) are float32r (FP22-truncated fp32) which streams at
full PE rate for moving dims >= 256; stationary weights are bf16 (halves
weight DMA, enables fast-weight-load). The host scatters per-core outputs
back to the original token order. Compute per core is ~cap/512 of the ideal
balanced load (cap = max tokens routed to one expert), ~8x less than the
dense all-experts reference.

Weights are host-packed into the exact SBUF image layout so every weight DMA
moves multi-KB contiguous lines; w1 streams on the sync HWDGE ring while w2
streams on the scalar HWDGE ring.

Self-contained: shapes hardcoded from the problem spec.
"""

import math
from functools import lru_cache

import ml_dtypes
import numpy as np

import concourse.bacc as bacc
import concourse.mybir as mybir
import concourse.tile as tile
from concourse.bass_utils import run_bass_kernel_spmd

E = 8
B = 4096
D_IN = 512
D_FF = 2048
D_OUT = 512
GROUP_COL = 0

P = 128
D_T = D_IN // P   # 4  k-tiles for layer 1
F_T = D_FF // P   # 16 f-tiles (layer-1 out / layer-2 contraction)
O_T = D_OUT // P  # 4  o-tiles for layer 2
W1_G = 4          # w1 DMA granularity: f-tiles per DMA (slice 0 leads the ring)

F32 = mybir.dt.float32
F32R = mybir.dt.float32r
BF16 = mybir.dt.bfloat16

# walrus requires matmul operand dtypes to match when fp32/fp32r is involved,
# so weights and moving activations share one dtype: bf16 (fast weight load,
# half the DMA bytes) or float32r (better precision, full PE rate at N>=256).
ALL_BF16 = True
W_DT = A_DT = BF16 if ALL_BF16 else F32R
W_NP = ml_dtypes.bfloat16 if ALL_BF16 else np.float32


N_WARM = 18  # dependency-free scratch matmuls to lift the PE HAM clock
             # gate to 8/8 (2.4 GHz) while the pilot DMAs are in flight


def _emit(tc, nc, xT, w1, w2, b1t, b2t, yT, cap, n_chunks, chunk):
    relu = mybir.ActivationFunctionType.Relu
    ident = mybir.ActivationFunctionType.Identity
    from concourse.bass import _add_dep_helper

    with (
        tc.tile_pool(name="consts", bufs=1) as cpool,
        tc.tile_pool(name="acts", bufs=1) as apool,
        tc.tile_pool(name="yout", bufs=4) as ypool,
        tc.tile_pool(name="psum_h", bufs=4, space="PSUM") as ph,
        tc.tile_pool(name="psum_y", bufs=4, space="PSUM") as py,
    ):
        # ---- PE warm-up: scratch matmuls with no input dependencies. They
        # run during the input-DMA wait and hold the HAM activity window
        # busy, so the real stream starts at 2.4 GHz instead of 1.2 GHz.
        warm_w = cpool.tile([P, P], W_DT)
        warm_x = cpool.tile([P, chunk], A_DT)
        nc.gpsimd.memset(warm_w[:], 0.0)
        nc.gpsimd.memset(warm_x[:], 0.0)
        warm_p = py.tile([P, chunk], F32, name="warm_p", tag="yp")
        for _ in range(N_WARM):
            nc.tensor.matmul(warm_p[:], warm_w[:], warm_x[:])

        # ---- input DMAs. The sync-issued HWDGE ring sustains ~300+ GB/s;
        # the scalar-issued ring only ~40-150 GB/s (worse while sync is
        # busy). So the whole critical path rides the sync ring, FIFO in
        # exactly the order the PE consumes it: xT, w1 slices smallest-
        # first, then half of w2. The scalar ring carries only soft-
        # deadline bytes (biases, w2 k2/k3) issued up front -- nothing is
        # gated, ring assignment + FIFO do all the pacing.
        w1_sb = cpool.tile([P, F_T, D_T, P], W_DT)   # [p, i, j, c]
        w2_sb = cpool.tile([P, O_T, F_T, P], W_DT)   # [p, k, i, c]
        xT_sb = apool.tile([P, n_chunks, D_T, chunk], A_DT)

        b1_sb = cpool.tile([P, F_T], F32)
        b2_sb = cpool.tile([P, O_T], F32)
        nc.scalar.dma_start(b1_sb[:], b1t.ap())
        nc.scalar.dma_start(b2_sb[:], b2t.ap())
        for c in range(n_chunks):
            nc.sync.dma_start(xT_sb[:, c], xT.ap()[c])
        for lo, hi in [(0, 1), (1, 3), (3, 6), (6, 10), (10, 16)]:
            nc.sync.dma_start(w1_sb[:, lo:hi], w1.ap()[:, lo:hi])
        nc.sync.dma_start(w2_sb[:, 0], w2.ap()[:, 0])
        nc.sync.dma_start(w2_sb[:, 1], w2.ap()[:, 1])
        nc.scalar.dma_start(w2_sb[:, 2], w2.ap()[:, 2])
        nc.scalar.dma_start(w2_sb[:, 3], w2.ap()[:, 3])

        hT_sb = apool.tile([P, F_T, cap], A_DT)

        first_mm = None
        # ---- layer 1: hT[f, c] = relu(sum_d W1[d, f] xT[d, c] + b1[f])
        # chunk-interleaved so each w1 i-slice is consumed over 2x the time
        for i in range(F_T):
            for c in range(n_chunks):
                cs = slice(c * chunk, (c + 1) * chunk)
                hp = ph.tile([P, chunk], F32, name=f"hp_{i}_{c}", tag="hp")
                for j in range(D_T):
                    mm = nc.tensor.matmul(
                        hp[:],
                        w1_sb[:, i, j, :],
                        xT_sb[:, c, j, :],
                        start=(j == 0),
                        stop=(j == D_T - 1),
                    )
                    if first_mm is None:
                        first_mm = mm
                nc.scalar.activation(
                    hT_sb[:, i, cs], hp[:], relu, bias=b1_sb[:, i : i + 1]
                )
        # ---- layer 2: yT[o, c] = sum_f W2[f, o] hT[f, c] + b2[o]
        # output DMAs split across both HWDGE rings to shorten the tail
        for k in range(O_T):
            for c in range(n_chunks):
                cs = slice(c * chunk, (c + 1) * chunk)
                yp = py.tile([P, chunk], F32, name=f"yp_{k}_{c}", tag="yp")
                for i in range(F_T):
                    nc.tensor.matmul(
                        yp[:],
                        w2_sb[:, k, i, :],
                        hT_sb[:, i, cs],
                        start=(i == 0),
                        stop=(i == F_T - 1),
                    )
                yo = ypool.tile([P, chunk], F32, name=f"yo_{k}_{c}", tag="yo")
                half = chunk // 2
                c0 = c * chunk
                last = k == O_T - 1 and c == n_chunks - 1
                if not last:
                    nc.scalar.activation(yo[:], yp[:], ident, bias=b2_sb[:, k : k + 1])
                    nc.sync.dma_start(
                        yT[k * P : (k + 1) * P, c0 : c0 + half], yo[:, 0:half]
                    )
                    nc.sync.dma_start(
                        yT[k * P : (k + 1) * P, c0 + half : c0 + chunk],
                        yo[:, half:chunk],
                    )
                else:
                    # final group: two half-width ACTs so the last bytes can
                    # leave (on both rings) as early as possible
                    nc.scalar.activation(
                        yo[:, 0:half], yp[:, 0:half], ident, bias=b2_sb[:, k : k + 1]
                    )
                    nc.sync.dma_start(
                        yT[k * P : (k + 1) * P, c0 : c0 + half], yo[:, 0:half]
                    )
                    nc.scalar.activation(
                        yo[:, half:chunk], yp[:, half:chunk], ident,
                        bias=b2_sb[:, k : k + 1],
                    )
                    nc.sync.dma_start(
                        yT[k * P : (k + 1) * P, c0 + half : c0 + chunk],
                        yo[:, half:chunk],
                    )


@lru_cache(maxsize=4)
def _build_nc(cap, n_chunks, chunk):
    nc = bacc.Bacc("TRN2", target_bir_lowering=False, debug=False, num_devices=E)
    xT = nc.dram_tensor("xT", [n_chunks, P, D_T, chunk], A_DT, kind="ExternalInput")
    w1 = nc.dram_tensor("w1", [P, F_T, D_T, P], W_DT, kind="ExternalInput")
    w2 = nc.dram_tensor("w2", [P, O_T, F_T, P], W_DT, kind="ExternalInput")
    b1t = nc.dram_tensor("b1t", [P, F_T], F32, kind="ExternalInput")
    b2t = nc.dram_tensor("b2t", [P, O_T], F32, kind="ExternalInput")
    yT = nc.dram_tensor("yT", [D_OUT, cap], F32, kind="ExternalOutput")
    with tile.TileContext(nc) as tc:
        _emit(tc, nc, xT, w1, w2, b1t, b2t, yT, cap, n_chunks, chunk)
    nc.compile()
    return nc


def _plan_capacity(max_count):
    cap0 = max(int(max_count), 16)
    n_chunks = max(1, math.ceil(cap0 / 512))
    chunk = math.ceil(cap0 / (n_chunks * 2)) * 2
    return n_chunks * chunk, n_chunks, chunk


def _pack_w1(W1e):
    # w1img[p, i, j, c] = W1e[j*128 + p, i*128 + c]
    return np.ascontiguousarray(
        W1e.reshape(D_T, P, F_T, P).transpose(1, 2, 0, 3).astype(W_NP)
    )


def _pack_w2(W2e):
    # w2img[p, k, i, c] = W2e[i*128 + p, k*128 + c]
    return np.ascontiguousarray(
        W2e.reshape(F_T, P, O_T, P).transpose(1, 2, 0, 3).astype(W_NP)
    )


def _shard(x, groups, W1, b1, W2, b2):
    idx = np.asarray(groups)[:, GROUP_COL].astype(np.int64)
    order = np.argsort(idx, kind="stable")
    counts = np.bincount(idx, minlength=E)
    cap, n_chunks, chunk = _plan_capacity(counts.max())
    offs = np.concatenate([[0], np.cumsum(counts)])

    x = np.asarray(x, dtype=np.float32)
    W1 = np.asarray(W1, dtype=np.float32)
    b1 = np.asarray(b1, dtype=np.float32)
    W2 = np.asarray(W2, dtype=np.float32)
    b2 = np.asarray(b2, dtype=np.float32)

    in_maps, tok_ids = [], []
    for e in range(E):
        ids = order[offs[e] : offs[e + 1]]
        tok_ids.append(ids)
        xT = np.zeros((D_IN, cap), np.float32)
        xT[:, : len(ids)] = x[ids].T
        # pack each chunk to its SBUF image [p, j, c] so every chunk DMA
        # moves multi-KB contiguous lines on both sides
        xT = np.ascontiguousarray(
            xT.reshape(D_T, P, n_chunks, chunk)
            .transpose(2, 1, 0, 3)
            .astype(W_NP)
        )
        in_maps.append(
            {
                "xT": xT,
                "w1": _pack_w1(W1[e]),
                "w2": _pack_w2(W2[e]),
                "b1t": np.ascontiguousarray(b1[e].reshape(F_T, P).T),
                "b2t": np.ascontiguousarray(b2[e].reshape(O_T, P).T),
            }
        )
    return in_maps, tok_ids, counts, cap, n_chunks, chunk


def _run(x, groups, W1, b1, W2, b2, trace=False, **spmd_kwargs):
    in_maps, tok_ids, counts, cap, n_chunks, chunk = _shard(x, groups, W1, b1, W2, b2)
    nc = _build_nc(cap, n_chunks, chunk)
    res = run_bass_kernel_spmd(
        nc, in_maps, core_ids=list(range(E)), trace=trace, **spmd_kwargs
    )
    out = np.zeros((B, D_OUT), np.float32)
    for e in range(E):
        yTe = res.results[e]["yT"]
        out[tok_ids[e]] = yTe[:, : counts[e]].T
    return out, res


def kernel(x, groups, W1, b1, W2, b2):
    out, _ = _run(x, groups, W1, b1, W2, b2)
    return out


# revision 15
# speedup vs baseline: 1.1488x; 1.0706x over previous
"""nn_GateMulti — MoE routing (8 experts, one-hot gate) on 8 TRN2 NeuronCores.

Strategy: expert-parallel. The gate is exactly one-hot on groups[:, 0], so
each token needs exactly one expert's MLP. Host-side "all-to-all": sort the
4096 tokens by expert id, pad each expert's token set to a common capacity,
and hand core e exactly expert e's tokens (transposed) plus expert e's
weights. Each core then runs a dense 2-layer MLP:

    yT = W2.T @ relu(W1.T @ xT + b1) + b2        (feature-major layout)

Moving operands (xT, hT) are float32r (FP22-truncated fp32) which streams at
full PE rate for moving dims >= 256; stationary weights are bf16 (halves
weight DMA, enables fast-weight-load). The host scatters per-core outputs
back to the original token order. Compute per core is ~cap/512 of the ideal
balanced load (cap = max tokens routed to one expert), ~8x less than the
dense all-experts reference.

Weights are host-packed into the exact SBUF image layout so every weight DMA
moves multi-KB contiguous lines; w1 streams on the sync HWDGE ring while w2
streams on the scalar HWDGE ring.

Self-contained: shapes hardcoded from the problem spec.
"""

import math
from functools import lru_cache

import ml_dtypes
import numpy as np

import concourse.bacc as bacc
import concourse.mybir as mybir
import concourse.tile as tile
from concourse.bass_utils import run_bass_kernel_spmd

E = 8
B = 4096
D_IN = 512
D_FF = 2048
D_OUT = 512
GROUP_COL = 0

P = 128
D_T = D_IN // P   # 4  k-tiles for layer 1
F_T = D_FF // P   # 16 f-tiles (layer-1 out / layer-2 contraction)
O_T = D_OUT // P  # 4  o-tiles for layer 2
W1_G = 4          # w1 DMA granularity: f-tiles per DMA (slice 0 leads the ring)

F32 = mybir.dt.float32
F32R = mybir.dt.float32r
BF16 = mybir.dt.bfloat16

# walrus requires matmul operand dtypes to match when fp32/fp32r is involved,
# so weights and moving activations share one dtype: bf16 (fast weight load,
# half the DMA bytes) or float32r (better precision, full PE rate at N>=256).
ALL_BF16 = True
W_DT = A_DT = BF16 if ALL_BF16 else F32R
W_NP = ml_dtypes.bfloat16 if ALL_BF16 else np.float32


N_WARM = 22  # dependency-free scratch matmuls to lift the PE HAM clock
             # gate to 8/8 (2.4 GHz) while the pilot DMAs are in flight


def _emit(tc, nc, xT, w1, w2, b1t, b2t, yT, cap, n_chunks, chunk):
    relu = mybir.ActivationFunctionType.Relu
    ident = mybir.ActivationFunctionType.Identity
    from concourse.bass import _add_dep_helper

    with (
        tc.tile_pool(name="consts", bufs=1) as cpool,
        tc.tile_pool(name="acts", bufs=1) as apool,
        tc.tile_pool(name="yout", bufs=4) as ypool,
        tc.tile_pool(name="psum_h", bufs=4, space="PSUM") as ph,
        tc.tile_pool(name="psum_y", bufs=4, space="PSUM") as py,
    ):
        # ---- PE warm-up: scratch matmuls with no input dependencies. They
        # run during the input-DMA wait and hold the HAM activity window
        # busy, so the real stream starts at 2.4 GHz instead of 1.2 GHz.
        warm_w = cpool.tile([P, P], W_DT)
        warm_x = cpool.tile([P, chunk], A_DT)
        nc.gpsimd.memset(warm_w[:], 0.0)
        nc.gpsimd.memset(warm_x[:], 0.0)
        warm_p = py.tile([P, chunk], F32, name="warm_p", tag="yp")
        for _ in range(N_WARM):
            nc.tensor.matmul(warm_p[:], warm_w[:], warm_x[:])

        # ---- input DMAs. Early-kernel DMA bandwidth is scarce (~150-250
        # GB/s ramping) and the rings fair-share per packet, so ONLY the
        # critical-path bytes may flow before the PE starts: the sync ring
        # carries xT then the w1 slices in consumption order (FIFO paces
        # them), the scalar ring carries just b1 (then runs the ACTs). The
        # w2 + b2 bulk is issued on sync but gated behind the first real
        # matmul so it cannot steal pilot bandwidth, and it never blocks
        # the scalar/ACT queue.
        w1_sb = cpool.tile([P, F_T, D_T, P], W_DT)   # [p, i, j, c]
        w2_sb = cpool.tile([P, O_T, F_T, P], W_DT)   # [p, k, i, c]
        xT_sb = apool.tile([P, n_chunks, D_T, chunk], A_DT)

        b1_sb = cpool.tile([P, F_T], F32)
        nc.scalar.dma_start(b1_sb[:], b1t.ap())
        for c in range(n_chunks):
            nc.sync.dma_start(xT_sb[:, c], xT.ap()[c])
        for lo, hi in [(0, 2), (2, 5), (5, 9), (9, 13), (13, 16)]:
            nc.sync.dma_start(w1_sb[:, lo:hi], w1.ap()[:, lo:hi])
        bulk = []
        for k in range(O_T):
            bulk.append(nc.sync.dma_start(w2_sb[:, k], w2.ap()[:, k]))
        b2_sb = cpool.tile([P, O_T], F32)
        bulk.append(nc.sync.dma_start(b2_sb[:], b2t.ap()))

        hT_sb = apool.tile([P, F_T, cap], A_DT)

        first_mm = None
        # ---- layer 1: hT[f, c] = relu(sum_d W1[d, f] xT[d, c] + b1[f])
        # chunk-interleaved so each w1 i-slice is consumed over 2x the time
        for i in range(F_T):
            for c in range(n_chunks):
                cs = slice(c * chunk, (c + 1) * chunk)
                hp = ph.tile([P, chunk], F32, name=f"hp_{i}_{c}", tag="hp")
                for j in range(D_T):
                    mm = nc.tensor.matmul(
                        hp[:],
                        w1_sb[:, i, j, :],
                        xT_sb[:, c, j, :],
                        start=(j == 0),
                        stop=(j == D_T - 1),
                    )
                    if first_mm is None:
                        first_mm = mm
                nc.scalar.activation(
                    hT_sb[:, i, cs], hp[:], relu, bias=b1_sb[:, i : i + 1]
                )
        for dd in bulk:
            _add_dep_helper(
                dd.ins, first_mm.ins, sync=True, reason="hold w2 until pilot landed"
            )
        # ---- layer 2: yT[o, c] = sum_f W2[f, o] hT[f, c] + b2[o]
        # output DMAs split across both HWDGE rings to shorten the tail
        for k in range(O_T):
            for c in range(n_chunks):
                cs = slice(c * chunk, (c + 1) * chunk)
                yp = py.tile([P, chunk], F32, name=f"yp_{k}_{c}", tag="yp")
                for i in range(F_T):
                    nc.tensor.matmul(
                        yp[:],
                        w2_sb[:, k, i, :],
                        hT_sb[:, i, cs],
                        start=(i == 0),
                        stop=(i == F_T - 1),
                    )
                yo = ypool.tile([P, chunk], F32, name=f"yo_{k}_{c}", tag="yo")
                half = chunk // 2
                c0 = c * chunk
                last = k == O_T - 1 and c == n_chunks - 1
                if not last:
                    nc.scalar.activation(yo[:], yp[:], ident, bias=b2_sb[:, k : k + 1])
                    nc.sync.dma_start(
                        yT[k * P : (k + 1) * P, c0 : c0 + half], yo[:, 0:half]
                    )
                    nc.sync.dma_start(
                        yT[k * P : (k + 1) * P, c0 + half : c0 + chunk],
                        yo[:, half:chunk],
                    )
                else:
                    # final group: two half-width ACTs so the last bytes can
                    # leave (on both rings) as early as possible
                    nc.scalar.activation(
                        yo[:, 0:half], yp[:, 0:half], ident, bias=b2_sb[:, k : k + 1]
                    )
                    nc.sync.dma_start(
                        yT[k * P : (k + 1) * P, c0 : c0 + half], yo[:, 0:half]
                    )
                    nc.scalar.activation(
                        yo[:, half:chunk], yp[:, half:chunk], ident,
                        bias=b2_sb[:, k : k + 1],
                    )
                    nc.sync.dma_start(
                        yT[k * P : (k + 1) * P, c0 + half : c0 + chunk],
                        yo[:, half:chunk],
                    )


@lru_cache(maxsize=4)
def _build_nc(cap, n_chunks, chunk):
    nc = bacc.Bacc("TRN2", target_bir_lowering=False, debug=False, num_devices=E)
    xT = nc.dram_tensor("xT", [n_chunks, P, D_T, chunk], A_DT, kind="ExternalInput")
    w1 = nc.dram_tensor("w1", [P, F_T, D_T, P], W_DT, kind="ExternalInput")
    w2 = nc.dram_tensor("w2", [P, O_T, F_T, P], W_DT, kind="ExternalInput")
    b1t = nc.dram_tensor("b1t", [P, F_T], F32, kind="ExternalInput")
    b2t = nc.dram_tensor("b2t", [P, O_T], F32, kind="ExternalInput")
    yT = nc.dram_tensor("yT", [D_OUT, cap], F32, kind="ExternalOutput")
    with tile.TileContext(nc) as tc:
        _emit(tc, nc, xT, w1, w2, b1t, b2t, yT, cap, n_chunks, chunk)
    nc.compile()
    return nc


def _plan_capacity(max_count):
    cap0 = max(int(max_count), 16)
    n_chunks = max(1, math.ceil(cap0 / 512))
    chunk = math.ceil(cap0 / (n_chunks * 2)) * 2
    return n_chunks * chunk, n_chunks, chunk


def _pack_w1(W1e):
    # w1img[p, i, j, c] = W1e[j*128 + p, i*128 + c]
    return np.ascontiguousarray(
        W1e.reshape(D_T, P, F_T, P).transpose(1, 2, 0, 3).astype(W_NP)
    )


def _pack_w2(W2e):
    # w2img[p, k, i, c] = W2e[i*128 + p, k*128 + c]
    return np.ascontiguousarray(
        W2e.reshape(F_T, P, O_T, P).transpose(1, 2, 0, 3).astype(W_NP)
    )


def _shard(x, groups, W1, b1, W2, b2):
    idx = np.asarray(groups)[:, GROUP_COL].astype(np.int64)
    order = np.argsort(idx, kind="stable")
    counts = np.bincount(idx, minlength=E)
    cap, n_chunks, chunk = _plan_capacity(counts.max())
    offs = np.concatenate([[0], np.cumsum(counts)])

    x = np.asarray(x, dtype=np.float32)
    W1 = np.asarray(W1, dtype=np.float32)
    b1 = np.asarray(b1, dtype=np.float32)
    W2 = np.asarray(W2, dtype=np.float32)
    b2 = np.asarray(b2, dtype=np.float32)

    in_maps, tok_ids = [], []
    for e in range(E):
        ids = order[offs[e] : offs[e + 1]]
        tok_ids.append(ids)
        xT = np.zeros((D_IN, cap), np.float32)
        xT[:, : len(ids)] = x[ids].T
        # pack each chunk to its SBUF image [p, j, c] so every chunk DMA
        # moves multi-KB contiguous lines on both sides
        xT = np.ascontiguousarray(
            xT.reshape(D_T, P, n_chunks, chunk)
            .transpose(2, 1, 0, 3)
            .astype(W_NP)
        )
        in_maps.append(
            {
                "xT": xT,
                "w1": _pack_w1(W1[e]),
                "w2": _pack_w2(W2[e]),
                "b1t": np.ascontiguousarray(b1[e].reshape(F_T, P).T),
                "b2t": np.ascontiguousarray(b2[e].reshape(O_T, P).T),
            }
        )
    return in_maps, tok_ids, counts, cap, n_chunks, chunk


def _run(x, groups, W1, b1, W2, b2, trace=False, **spmd_kwargs):
    in_maps, tok_ids, counts, cap, n_chunks, chunk = _shard(x, groups, W1, b1, W2, b2)
    nc = _build_nc(cap, n_chunks, chunk)
    res = run_bass_kernel_spmd(
        nc, in_maps, core_ids=list(range(E)), trace=trace, **spmd_kwargs
    )
    out = np.zeros((B, D_OUT), np.float32)
    for e in range(E):
        yTe = res.results[e]["yT"]
        out[tok_ids[e]] = yTe[:, : counts[e]].T
    return out, res


def kernel(x, groups, W1, b1, W2, b2):
    out, _ = _run(x, groups, W1, b1, W2, b2)
    return out
